# revision 1
# baseline (speedup 1.0000x reference)
"""Trainium2 Bass kernel for BlittingStrokeModel (AA polyline rasterization).

Reference semantics: for each batch item, rasterize 16 AA line segments
(trajectory knots) onto a zero canvas via a point-to-segment distance field:
    dist = point-to-segment distance
    cov  = clip(line_width + 0.5 - dist, 0, 1)
    out  = max over segments, broadcast to 3 channels.

Device formulation (exact up to the reference's 1e-8/1e-12 epsilons). With
s = 1/sqrt(dd2), dd2 = dx^2+dy^2, dn2 = dd2/2:
    w   = (dx*x + dy*y - c0 - dn2) * s        # scaled, recentred dot product
    E   = relu(|w| - dn2*s)                   # segment-clamp excess / sqrt(dd2)
    Pp  = (dy*x - dx*y + cP) * s              # perpendicular line distance
    dist^2 = Pp^2 + E^2
    M   = min over segments of dist^2
    cov = clip(L + 0.5 - sqrt(M), 0, 1)
Max over segments of cov == cov(min dist) since cov is monotone in dist.

Per (segment, 128-row stripe) the engine split is:
    ACT: At = Abs(x*dxs + cdw)     [plane + abs]
    V/ACT: E = relu(At - dn2s)     [assignment balances engine load]
    V:   M' = min((aP*x+bP)^2 + E^2, M)   [one fused custom DVE op; the
         x plane comes from the DVE Idx generator, so Src1 carries M]
Stripes are emitted round-robin with two min-chains each, giving the Tile
scheduler 8 independent chains so no engine starves at the kernel tail.

Input-specialized program structure: host geometry (fp64, conservative
margins) decides per (core, segment, stripe) whether the segment can
influence the stripe at all (skip otherwise) and whether its endpoint-cap
term can matter there (drop the At/E ops and feed E=0 otherwise).  All 8
cores run one SPMD program whose per-stripe slot counts are the max over
cores; cores with fewer jobs pad with neutral coefficients (d2 = 1e12).
Programs are cached per structure; the custom DVE ops are registered at
runtime so this file is self-contained.

Sharding: data-parallel over batch, one image per NeuronCore (8 cores).
The output does not depend on the image *values*, so images never touch
the device; only tiny per-segment coefficient tables are uploaded.
"""

import numpy as np
from contextlib import ExitStack

B, C, H, W = 8, 3, 512, 512
K = 17
NSEG = K - 1
P = 128
NSTRIPE = H // P  # 4
MARG = 1.0  # conservative skip margin in pixels (fp32 error << 1e-2)

_state = {}


# --------------------------------------------------------------------------
# custom DVE ops
# --------------------------------------------------------------------------

def _register_dve_op(name, spec):
    import concourse.dve_ops as dve_ops
    from concourse.dve_ops import DveOp, OPS, _SUB_OPCODE_FOR_NAME, _CUSTOM_DVE_ROW_BASE
    from concourse.dve_spec import lower, _has_src1
    from concourse.dve_uop import DveOpSpec
    from concourse.dve_table_gen import dve_ver_for

    if name in _SUB_OPCODE_FOR_NAME:
        return next(o for o in OPS if o.name == name)
    row = _CUSTOM_DVE_ROW_BASE + len(OPS)
    assert row < 0x20
    _SUB_OPCODE_FOR_NAME[name] = row
    ver = dve_ver_for("TRN2")
    tmp = DveOpSpec(
        name=name, opcode=row, uops=lower(spec, ver=ver), rd1_en=_has_src1(spec)
    )
    op = DveOp(name, spec, subdim=False, uops_sha={ver: tmp.sha(ver)})
    OPS.append(op)
    dve_ops.CUSTOM_DVE_SPECS[name] = spec
    return op


def _get_dve_ops():
    if "ops" in _state:
        return _state["ops"]
    from concourse.dve_spec import (
        Spec, Src0, Src1, C0, C1, sq, minn, maxx, Idx, Zero, One,
    )

    def _idx(in0):
        return np.arange(in0.shape[-1], dtype=np.float32)[None, :]

    d2min = _register_dve_op(
        "STROKE_D2MIN_ANT",
        Spec(
            body=minn(sq(Idx * C0 + C1) + sq(Src0), Src1),
            reference=lambda in0, in1, s0, s1, imm2: np.minimum(
                (_idx(in0) * s0 + s1) ** 2 + in0.astype(np.float32) ** 2, in1
            ).astype(np.float32),
        ),
    )
    d2first = _register_dve_op(
        "STROKE_D2_ANT",
        Spec(
            body=sq(Idx * C0 + C1) + sq(Src0),
            reference=lambda in0, in1, s0, s1, imm2: (
                (_idx(in0) * s0 + s1) ** 2 + in0.astype(np.float32) ** 2
            ).astype(np.float32),
        ),
    )
    clip = _register_dve_op(
        "STROKE_CLIP_ANT",
        Spec(
            body=minn(maxx(C0 - Src0, Zero), One),
            reference=lambda in0, in1, s0, s1, imm2: np.minimum(
                np.maximum(s0 - in0.astype(np.float32), 0.0), 1.0
            ).astype(np.float32),
        ),
    )
    # line-only variants: x comes from Src0 (= xt tile) instead of the Idx
    # scan, so these lower to a single uOp pass (the Idx ops need two)
    lmin = _register_dve_op(
        "STROKE_LD2MIN_ANT",
        Spec(
            body=minn(sq(Src0 * C0 + C1), Src1),
            reference=lambda in0, in1, s0, s1, imm2: np.minimum(
                (in0.astype(np.float32) * s0 + s1) ** 2, in1
            ).astype(np.float32),
        ),
    )
    lfirst = _register_dve_op(
        "STROKE_LD2_ANT",
        Spec(
            body=sq(Src0 * C0 + C1),
            reference=lambda in0, in1, s0, s1, imm2: (
                (in0.astype(np.float32) * s0 + s1) ** 2
            ).astype(np.float32),
        ),
    )
    _state["ops"] = (d2min, d2first, clip, lmin, lfirst)
    return _state["ops"]


# --------------------------------------------------------------------------
# host geometry: which (segment, stripe) pairs can matter, per core
# --------------------------------------------------------------------------

def _segments(xy):
    """Guarded segment endpoints/deltas (fp64). xy: [K, 2]."""
    p0, p1 = xy[:-1].copy(), xy[1:].copy()
    d = p1 - p0
    degen = (d[:, 0] ** 2 + d[:, 1] ** 2) < 1e-12
    d[degen, 0] = 1e-6
    p1 = p0 + d
    return p0, p1, d


def _seg_rect_dist(p0, p1, ylo, yhi):
    """Distance from segment (p0,p1) to rect [0, W-1] x [ylo, yhi]."""
    def pt_in_rect(p):
        return (0.0 <= p[0] <= W - 1) and (ylo <= p[1] <= yhi)

    if pt_in_rect(p0) or pt_in_rect(p1):
        return 0.0

    def ptseg(p, s0, s1):
        d = s1 - s0
        dd = float(d @ d)
        if dd < 1e-18:
            return float(np.hypot(*(p - s0)))
        t = min(1.0, max(0.0, float((p - s0) @ d) / dd))
        return float(np.hypot(*(p - s0 - t * d)))

    def ccw(A, B, C):
        return (C[1] - A[1]) * (B[0] - A[0]) > (B[1] - A[1]) * (C[0] - A[0])

    def inter(A, B, C, D):
        return ccw(A, C, D) != ccw(B, C, D) and ccw(A, B, C) != ccw(A, B, D)

    corners = [
        np.array([0.0, ylo]), np.array([W - 1.0, ylo]),
        np.array([W - 1.0, yhi]), np.array([0.0, yhi]),
    ]
    best = np.inf
    for i in range(4):
        b0, b1 = corners[i], corners[(i + 1) % 4]
        if inter(p0, p1, b0, b1):
            return 0.0
        best = min(
            best,
            ptseg(p0, b0, b1), ptseg(p1, b0, b1),
            ptseg(b0, p0, p1), ptseg(b1, p0, p1),
        )
    return best


def _plan(trajectories, line_width):
    """Decide kept jobs and cap-need per (core, stripe); build the SPMD
    union structure and per-core slot assignments."""
    thr = float(np.asarray(line_width).item()) + 0.5
    xy = np.asarray(trajectories, dtype=np.float64)[:, :, 1:3]
    nb = xy.shape[0]
    R = thr + MARG
    FAR = 1500.0

    # jobs[b][T] = list of (seg, needs_cap) — cap-needing first
    jobs = [[[] for _ in range(NSTRIPE)] for _ in range(nb)]
    for b in range(nb):
        p0a, p1a, da = _segments(xy[b])
        for T in range(NSTRIPE):
            ylo, yhi = T * P + 0.0, T * P + P - 1.0
            full, line = [], []
            for s in range(NSEG):
                p0, p1, d = p0a[s], p1a[s], da[s]
                if _seg_rect_dist(p0, p1, ylo, yhi) > R:
                    continue
                dirv = d / max(float(np.hypot(*d)), 1e-9)
                cap = (
                    _seg_rect_dist(p0, p0 - dirv * FAR, ylo, yhi) <= R
                    or _seg_rect_dist(p1, p1 + dirv * FAR, ylo, yhi) <= R
                )
                (full if cap else line).append((s, cap))
            jobs[b][T] = line + full  # line-only jobs first (no At/E dep)

    # Decouple stripes from images: bin-pack all (image, stripe) pairs
    # across the cores (LPT) so per-core load equalizes — the stripe
    # identity lives entirely in host coefficients + output addressing.
    pairs = sorted(
        (
            (len(jobs[b][T]), sum(1 for _, cp in jobs[b][T] if cp), b, T)
            for b in range(nb)
            for T in range(NSTRIPE)
        ),
        reverse=True,
    )
    cores = [[] for _ in range(nb)]
    loads = [0] * nb
    for njp, ncp, b, T in pairs:
        cand = [c for c in range(nb) if len(cores[c]) < NSTRIPE]
        i = min(cand, key=lambda c: loads[c])
        cores[i].append((njp, ncp, b, T))
        loads[i] += njp
    for c in cores:
        c.sort(reverse=True)
    assign = [
        [(b, T, jobs[b][T]) for _, _, b, T in cores[c]] for c in range(nb)
    ]
    nj = tuple(
        max(1, max(cores[c][k][0] for c in range(nb))) for k in range(NSTRIPE)
    )
    ncap = tuple(
        max(cores[c][k][1] for c in range(nb)) for k in range(NSTRIPE)
    )
    # E-op engine split: balance V vs ACT load (costs in ns per op)
    nslot, ncaps = sum(nj), sum(ncap)
    x = int(round((800 * nslot - 400 * ncaps + 3800) / 1300.0))
    x = max(0, min(ncaps, x))
    # full-capable slots are the LAST ncap[T] of each stripe
    eact = []
    seen = 0
    for T in range(NSTRIPE):
        for j in range(nj[T]):
            if j >= nj[T] - ncap[T]:
                eact.append(seen < x)
                seen += 1
            else:
                eact.append(False)
    struct = (nj, ncap, tuple(eact))
    return struct, assign, thr


# --------------------------------------------------------------------------
# program build (per structure, cached)
# --------------------------------------------------------------------------

def _build_program(struct):
    import concourse.tile as tile
    from concourse import bacc, mybir

    dt = mybir.dt
    op = mybir.AluOpType
    af = mybir.ActivationFunctionType
    d2min_op, d2first_op, clip_op, lmin_op, lfirst_op = _get_dve_ops()
    nj, ncap, eact = struct
    nslot = sum(nj)

    nc = bacc.Bacc("TRN2", target_bir_lowering=False, debug=False)
    xt_d = nc.dram_tensor("xt", [P, W], dt.float32, kind="ExternalInput").ap()
    # per-slot scalars: [dxs, aP, dn2s, ndn2s] *nslot + [thr]
    cs_d = nc.dram_tensor("cs", [P, 4 * nslot + 1], dt.float32, kind="ExternalInput").ap()
    cdw_d = nc.dram_tensor("cdw", [P, nslot], dt.float32, kind="ExternalInput").ap()
    cbp_d = nc.dram_tensor("cbp", [P, nslot], dt.float32, kind="ExternalInput").ap()
    # one [C, 128, W] block per stripe-slot; the host reassembles into images
    out_d = nc.dram_tensor(
        "out", [NSTRIPE, C, P, W], dt.float32, kind="ExternalOutput"
    ).ap()

    with tile.TileContext(nc) as tc, ExitStack() as ctx:
        const = ctx.enter_context(tc.tile_pool(name="const", bufs=1))
        xt = const.tile_from(xt_d)
        cs = const.tile_from(cs_d)
        cbp = const.tile_from(cbp_d)
        cdw = const.tile_from(cdw_d)
        Z = const.tile([P, W], dt.float32, name="Z")
        nc.gpsimd.memset(Z[:], 0.0)

        work = ctx.enter_context(tc.tile_pool(name="work", bufs=8))
        mpool = ctx.enter_context(tc.tile_pool(name="m", bufs=16))
        opool = ctx.enter_context(tc.tile_pool(name="o", bufs=3))

        # warm the ACT function tables while const DMAs are in flight
        wu = opool.tile([P, 8], dt.float32, name="wu")
        nc.vector.memset(wu[:], 0.0)
        wu2 = opool.tile([P, 8], dt.float32, name="wu2")
        nc.scalar.activation(wu2[:], wu[:], af.Abs)
        nc.scalar.activation(wu2[:], wu[:], af.Relu)
        nc.scalar.activation(wu2[:], wu[:], af.Sqrt)

        # round-robin the stripes' jobs so all four stripes finish together
        # (8 independent min-chains keep every engine fed through the tail)
        goff = [sum(nj[:T]) for T in range(NSTRIPE)]
        chains = [[None, None] for _ in range(NSTRIPE)]

        def emit_job(T, j):
            g = goff[T] + j
            c4 = 4 * g
            Mn = mpool.tile([P, W], dt.float32, tag="M", name=f"M{g}")
            ci = j % 2
            prev = chains[T][ci]
            if j >= nj[T] - ncap[T]:
                At = work.tile([P, W], dt.float32, tag="At", name=f"At{g}")
                nc.scalar.activation(
                    At[:], xt[:], af.Abs,
                    bias=cdw[:, g : g + 1], scale=cs[:, c4 : c4 + 1],
                )
                E = work.tile([P, W], dt.float32, tag="E", name=f"E{g}")
                if eact[g]:
                    nc.scalar.activation(
                        E[:], At[:], af.Relu, bias=cs[:, c4 + 3 : c4 + 4]
                    )
                else:
                    nc.vector.tensor_scalar(
                        E[:], At[:], cs[:, c4 + 2 : c4 + 3], 0.0,
                        op0=op.subtract, op1=op.max,
                    )
                if prev is None:
                    nc.vector._custom_dve(
                        d2first_op, out=Mn[:], in0=E[:],
                        s0=cs[:, c4 + 1 : c4 + 2], s1=cbp[:, g : g + 1],
                    )
                else:
                    nc.vector._custom_dve(
                        d2min_op, out=Mn[:], in0=E[:], in1=prev[:],
                        s0=cs[:, c4 + 1 : c4 + 2], s1=cbp[:, g : g + 1],
                    )
            else:
                # line-only job: x rides Src0 (xt) — single-uOp variants
                if prev is None:
                    nc.vector._custom_dve(
                        lfirst_op, out=Mn[:], in0=xt[:],
                        s0=cs[:, c4 + 1 : c4 + 2], s1=cbp[:, g : g + 1],
                    )
                else:
                    nc.vector._custom_dve(
                        lmin_op, out=Mn[:], in0=xt[:], in1=prev[:],
                        s0=cs[:, c4 + 1 : c4 + 2], s1=cbp[:, g : g + 1],
                    )
            chains[T][ci] = Mn

        def finalize_stripe(T):
            if chains[T][1] is not None:
                M = mpool.tile([P, W], dt.float32, tag="M", name=f"Mf{T}")
                nc.vector.tensor_tensor(
                    M[:], chains[T][0][:], chains[T][1][:], op=op.min
                )
            else:
                M = chains[T][0]
            dist = opool.tile([P, W], dt.float32, tag="dist", name=f"ds{T}")
            nc.scalar.activation(dist[:], M[:], af.Sqrt)
            # cov = clip(thr - dist, 0, 1) in one fused DVE op
            cov = opool.tile([P, W], dt.float32, tag="cov", name=f"cv{T}")
            nc.vector._custom_dve(
                clip_op, out=cov[:], in0=dist[:],
                s0=cs[:, 4 * nslot : 4 * nslot + 1],
            )
            for c in range(C):
                nc.sync.dma_start(out_d[T, c, :, :], cov[:])

        for j in range(max(nj)):
            for T in range(NSTRIPE):
                if j < nj[T]:
                    emit_job(T, j)
                    if j == nj[T] - 1:
                        finalize_stripe(T)

    nc.compile()
    return nc


# --------------------------------------------------------------------------
# host coefficient tables
# --------------------------------------------------------------------------

def _prep_inputs(trajectories, struct, assign, thr):
    nj, ncap, _ = struct
    nslot = sum(nj)
    xy = np.asarray(trajectories, dtype=np.float64)[:, :, 1:3]
    nb = xy.shape[0]
    xt = np.broadcast_to(np.arange(W, dtype=np.float64), (P, W)).astype(np.float32)
    yv = np.arange(H, dtype=np.float64).reshape(NSTRIPE, P)

    geo = {}
    for b in range(nb):
        p0a, p1a, da = _segments(xy[b])
        dx, dy = da[:, 0], da[:, 1]
        dd2 = dx * dx + dy * dy
        sq = 1.0 / np.sqrt(dd2)
        dn2 = dd2 / 2.0
        c0 = dx * p0a[:, 0] + dy * p0a[:, 1]
        cP = dx * p0a[:, 1] - dy * p0a[:, 0]
        geo[b] = (dx, dy, sq, dn2, c0, cP)

    in_maps = []
    for core in range(nb):
        cs = np.zeros((P, 4 * nslot + 1))
        cdw = np.zeros((P, nslot))
        cbp = np.zeros((P, nslot))
        g = 0
        for k in range(NSTRIPE):
            b, T, myjobs = assign[core][k]
            dx, dy, sq, dn2, c0, cP = geo[b]
            cap_jobs = [s for s, cap in myjobs if cap]
            line_jobs = [s for s, cap in myjobs if not cap]
            full_start = nj[k] - ncap[k]
            slots = [None] * nj[k]
            for i, s in enumerate(cap_jobs):
                slots[full_start + i] = (s, True)
            free = list(range(full_start)) + list(
                range(full_start + len(cap_jobs), nj[k])
            )
            for s, j in zip(line_jobs, free):
                slots[j] = (s, False)
            for j in range(nj[k]):
                c4 = 4 * g
                if slots[j] is not None:
                    s, iscap = slots[j]
                    cs[:, c4 + 0] = dx[s] * sq[s]
                    cs[:, c4 + 1] = dy[s] * sq[s]
                    # E = 0 unless this is a genuine cap job (t-clamp excess
                    # provably irrelevant in this stripe otherwise)
                    if iscap:
                        cs[:, c4 + 2] = dn2[s] * sq[s]
                        cs[:, c4 + 3] = -dn2[s] * sq[s]
                    else:
                        cs[:, c4 + 2] = 1e30
                        cs[:, c4 + 3] = -1e30
                    cdw[:, g] = (dy[s] * yv[T] - (c0[s] + dn2[s])) * sq[s]
                    cbp[:, g] = (-dx[s] * yv[T] + cP[s]) * sq[s]
                else:
                    # neutral padding: d2 = 1e12, E = 0
                    cs[:, c4 + 0] = 0.0
                    cs[:, c4 + 1] = 0.0
                    cs[:, c4 + 2] = 1e30
                    cs[:, c4 + 3] = -1e30
                    cdw[:, g] = 0.0
                    cbp[:, g] = 1e6
                g += 1
        cs[:, 4 * nslot] = thr

        in_maps.append(
            {
                "xt": xt,
                "cs": cs.astype(np.float32),
                "cdw": cdw.astype(np.float32),
                "cbp": cbp.astype(np.float32),
            }
        )
    return in_maps


def kernel(**inputs):
    from concourse.bass_utils import run_bass_kernel_spmd

    images = np.asarray(inputs["images"])
    trajectories = np.asarray(inputs["trajectories"])
    line_width = inputs["line_width"]
    assert images.shape == (B, C, H, W), images.shape

    struct, assign, thr = _plan(trajectories, line_width)
    progs = _state.setdefault("progs", {})
    if struct not in progs:
        progs[struct] = _build_program(struct)
    nc = progs[struct]

    in_maps = _prep_inputs(trajectories, struct, assign, thr)
    res = run_bass_kernel_spmd(nc, in_maps, list(range(B))).results
    out = np.empty((B, C, H, W), np.float32)
    for core in range(B):
        blk = res[core]["out"]  # [NSTRIPE, C, P, W]
        for k in range(NSTRIPE):
            b, T, _ = assign[core][k]
            out[b, :, T * P : (T + 1) * P, :] = blk[k]
    return out


if __name__ == "__main__":
    rng = np.random.default_rng(0)
    ins = {
        "images": rng.standard_normal((B, C, H, W)).astype(np.float32),
        "trajectories": np.concatenate(
            [
                np.broadcast_to(np.linspace(0, 1, K, dtype=np.float32), (B, K))[..., None],
                rng.uniform(0, W - 1, (B, K, 2)).astype(np.float32),
                np.ones((B, K, 1), np.float32),
            ],
            axis=-1,
        ),
        "line_width": 3,
    }
    out = kernel(**ins)
    print(out.shape, out.dtype, out.min(), out.max())



# revision 2
# speedup vs baseline: 1.1217x; 1.1217x over previous
"""Trainium2 Bass kernel v2 for BlittingStrokeModel (AA polyline rasterization).

Reference semantics: per batch item, stamp 16 AA segments onto a zero canvas:
    dist = point-to-segment distance
    cov  = clip(line_width + 0.5 - dist, 0, 1), max over segments, x3 channels.

v2 design (vs v1's full-width padded SPMD slots):
  * Exact per-core programs via tc.Switch(core_id, 8): every op covers only
    the columns its segment can influence (~19% of full width).
  * In-place min accumulation: per stripe a single M tile [128, 512] is
    memset to 1e12 and updated in place by column-ranged fused min ops.
  * Cap (endpoint-clamp) jobs use ONE fused custom DVE op:
        CAPHMIN:  M = min((Idx*C0+C1)^2 + relu(Src0 + imm2)^2, M)
    with Src0 = |w| from one ACT Abs op, imm2 = -h baked as an immediate
    (input-specialized program; cached per input).
        CAPAHMIN: same but Src0 = w signed (abs folded in-op), producer is a
    single V tensor_scalar affine — used when ACT is the bottleneck.
  * Line jobs: single fused LD2MIN (xt-driven, absolute x, no producer).
  * Finalize: ACT sqrt -> (dist - thr) on V (or relu(thr-dist) on ACT);
    the [0,1] clamp happens on host. Only ONE channel is written; host
    broadcasts to 3 channels (output is channel-identical).
  * Host does fp64 geometry: job detection per (image, stripe), exact
    endpoint-cap need via strip-vs-window polygon clipping, LPT stripe->core
    packing, greedy V/ACT load balancing with measured op costs.

Images never touch the device (output is independent of image values).
"""

import numpy as np
from contextlib import ExitStack

B, C, H, W = 8, 3, 512, 512
K = 17
NSEG = K - 1
P = 128
NSTRIPE = H // P  # 4
RMARG = 0.7   # job-range margin (px)
TAU_M = 0.6   # cap-test band margin (px)

# measured per-op costs (ns), back-to-back on HW
def _c_act(w): return 0.833 * w + 280.0
def _c_vcus(w): return 1.042 * w + 189.0
def _c_vts(w): return 0.521 * w + 129.0

_state = {}


# --------------------------------------------------------------------------
# custom DVE ops
# --------------------------------------------------------------------------

def _register_dve_op(name, spec):
    import concourse.dve_ops as dve_ops
    from concourse.dve_ops import DveOp, OPS, _SUB_OPCODE_FOR_NAME, _CUSTOM_DVE_ROW_BASE
    from concourse.dve_spec import lower, _has_src1
    from concourse.dve_uop import DveOpSpec
    from concourse.dve_table_gen import dve_ver_for

    if name in _SUB_OPCODE_FOR_NAME:
        return next(o for o in OPS if o.name == name)
    row = _CUSTOM_DVE_ROW_BASE + len(OPS)
    assert row < 0x20
    _SUB_OPCODE_FOR_NAME[name] = row
    ver = dve_ver_for("TRN2")
    tmp = DveOpSpec(
        name=name, opcode=row, uops=lower(spec, ver=ver), rd1_en=_has_src1(spec)
    )
    op = DveOp(name, spec, subdim=False, uops_sha={ver: tmp.sha(ver)})
    OPS.append(op)
    dve_ops.CUSTOM_DVE_SPECS[name] = spec
    return op


def _get_dve_ops():
    if "ops" in _state:
        return _state["ops"]
    from concourse.dve_spec import (
        Spec, Src0, Src1, C0, C1, C2, sq, minn, maxx, relu, Idx, Zero,
        Scan, Bin, AluOp,
    )

    def _idx1(in0):
        # scan(ADD, C0, init=C1) at element k equals C1 + C0*(k+1)
        return np.arange(in0.shape[-1], dtype=np.float32)[None, :] + 1.0

    _idx1v = _idx1

    lmin = _register_dve_op(
        "STROKE_LD2MIN_ANT",
        Spec(
            body=minn(sq(Src0 * C0 + C1), Src1),
            reference=lambda in0, in1, s0, s1, imm2: np.minimum(
                (in0.astype(np.float32) * s0 + s1) ** 2, in1
            ).astype(np.float32),
        ),
    )
    caphmin = _register_dve_op(
        "STROKE_CAPHSC_ANT",
        Spec(
            body=minn(
                sq(Scan(AluOp.ADD, C0, init=C1)) + sq(relu(Src0 + C2)), Src1
            ),
            reference=lambda in0, in1, s0, s1, imm2: np.minimum(
                (_idx1(in0) * s0 + s1) ** 2
                + np.maximum(in0.astype(np.float32) + imm2, 0.0) ** 2,
                in1,
            ).astype(np.float32),
        ),
    )
    e2v = _register_dve_op(
        "STROKE_E2V_ANT",
        Spec(
            body=sq(relu(Bin(AluOp.ABSOLUTE_VALUE, Src0 * C0 + C1, Zero) + C2)),
            reference=lambda in0, in1, s0, s1, imm2: (
                np.maximum(
                    np.abs(in0.astype(np.float32) * s0 + s1) + imm2, 0.0
                ) ** 2
            ).astype(np.float32),
        ),
    )
    sqimin = _register_dve_op(
        "STROKE_SQIMIN_ANT",
        Spec(
            body=minn(sq(Scan(AluOp.ADD, C0, init=C1)) + Src0, Src1),
            reference=lambda in0, in1, s0, s1, imm2: np.minimum(
                (_idx1(in0) * s0 + s1) ** 2 + in0.astype(np.float32) * 0
                + in0.astype(np.float32),
                in1,
            ).astype(np.float32),
        ),
    )
    _state["ops"] = (lmin, caphmin, e2v, sqimin)
    return _state["ops"]


# --------------------------------------------------------------------------
# host geometry
# --------------------------------------------------------------------------

def _segments(xy):
    """Guarded segment endpoints/deltas (fp64). xy: [K, 2]."""
    p0, p1 = xy[:-1].copy(), xy[1:].copy()
    d = p1 - p0
    degen = (d[:, 0] ** 2 + d[:, 1] ** 2) < 1e-12
    d[degen, 0] = 1e-6
    p1 = p0 + d
    return p0, p1, d


def _clip_poly(poly, a, b, c):
    """Keep the part of polygon with a*x + b*y <= c."""
    out = []
    n = len(poly)
    for i in range(n):
        p, q = poly[i], poly[(i + 1) % n]
        fp = a * p[0] + b * p[1] - c
        fq = a * q[0] + b * q[1] - c
        if fp <= 0:
            out.append(p)
        if (fp < 0) != (fq < 0) and fp != fq:
            t = fp / (fp - fq)
            out.append((p[0] + t * (q[0] - p[0]), p[1] + t * (q[1] - p[1])))
    return out


def _jobs_for(xy, thr):
    """Per stripe: list of (seg, x0, w, needs_cap). fp64 geometry."""
    R = thr + RMARG
    TAU = thr + TAU_M
    FAR = 2000.0
    p0a, p1a, da = _segments(xy)
    out = [[] for _ in range(NSTRIPE)]
    for T in range(NSTRIPE):
        ylo, yhi = T * P + 0.0, T * P + 127.0
        for s in range(NSEG):
            p0, p1, d = p0a[s], p1a[s], da[s]
            y0s, y1s = (p0[1], p1[1]) if p0[1] <= p1[1] else (p1[1], p0[1])
            if y1s < ylo - R or y0s > yhi + R:
                continue
            dy = d[1]
            if abs(dy) < 1e-12:
                t0c, t1c = 0.0, 1.0
            else:
                ta = (ylo - R - p0[1]) / dy
                tb = (yhi + R - p0[1]) / dy
                t0c, t1c = max(0.0, min(ta, tb)), min(1.0, max(ta, tb))
            if t0c > t1c:
                continue
            xs0 = p0[0] + t0c * d[0]
            xs1 = p0[0] + t1c * d[0]
            x0f = max(0.0, min(xs0, xs1) - R)
            x1f = min(W - 1.0, max(xs0, xs1) + R)
            if x1f < x0f:
                continue
            X0, X1 = int(np.floor(x0f)), int(np.ceil(x1f))
            w = X1 - X0 + 1
            # exact endpoint-cap need: does the beyond-endpoint strip
            # (width 2*TAU along the extension ray) intersect the window?
            L = float(np.hypot(*d))
            u = d / L
            nv = np.array([-u[1], u[0]])
            cap = False
            for pe, sgn in ((p1, 1.0), (p0, -1.0)):
                du = u * sgn
                poly = [
                    tuple(pe + TAU * nv), tuple(pe + FAR * du + TAU * nv),
                    tuple(pe + FAR * du - TAU * nv), tuple(pe - TAU * nv),
                ]
                poly = _clip_poly(poly, 1, 0, X1 + 0.5)
                if poly:
                    poly = _clip_poly(poly, -1, 0, -(X0 - 0.5))
                if poly:
                    poly = _clip_poly(poly, 0, 1, yhi + 0.5)
                if poly:
                    poly = _clip_poly(poly, 0, -1, -(ylo - 0.5))
                if poly:
                    cap = True
                    break
            out[T].append((s, X0, w, cap))
    return out


# --------------------------------------------------------------------------
# planning: stripe->core packing + per-core schedule & engine balance
# --------------------------------------------------------------------------

def _plan(trajectories, line_width):
    thr = float(np.asarray(line_width).item()) + 0.5
    xy = np.asarray(trajectories, dtype=np.float64)[:, :, 1:3]
    nb = xy.shape[0]

    jobs = {}
    for b in range(nb):
        jb = _jobs_for(xy[b], thr)
        for T in range(NSTRIPE):
            jobs[(b, T)] = jb[T]

    def stripe_cost(jl):
        c = 0.0
        for s, x0, w, cap in jl:
            c += _c_vcus(w) + (_c_act(w) if cap else 0.0) * 0.5
        return c

    pairs = sorted(jobs, key=lambda p: -stripe_cost(jobs[p]))
    cores = [[] for _ in range(8)]
    loads = [0.0] * 8
    for pr in pairs:
        cand = [c for c in range(8) if len(cores[c]) < NSTRIPE]
        i = min(cand, key=lambda c: loads[c])
        cores[i].append(pr)
        loads[i] += stripe_cost(jobs[pr])

    # per-core schedule: round-robin stripes, greedy engine choice
    arms = []     # structural, hashable
    fills = []    # per-core value-filling recipes
    assign = []   # per-core [(b, T)] per slot + finsub engine flags
    ncol_max = 1
    for core in range(8):
        slots = cores[core]
        # order slots by descending job count so round-robin staggers finishes
        slots = sorted(slots, key=lambda p: -len(jobs[p]))
        while len(slots) < NSTRIPE:
            slots.append(None)
        jl = [list(jobs[p]) if p is not None else [] for p in slots]
        # line jobs first (V-only, no ACT dep) then caps, wide first
        for l in jl:
            l.sort(key=lambda j: (j[3], -j[2]))
        items = []   # structural items
        fill = []    # (kind, col, values...) value recipes
        # seed with startup costs: ACT table load + warmup; V memset wait
        acc = {"V": 300.0, "ACT": 1900.0}
        col = 1      # col 0 = thr
        idx = [0] * NSTRIPE
        done = [False] * NSTRIPE
        finflags = [None] * NSTRIPE
        rr = 0
        while not all(done):
            # plain round-robin across unfinished stripes (4 independent
            # chains keep V fed; equal finish keeps the critical path short)
            while done[rr % NSTRIPE]:
                rr += 1
            k = rr % NSTRIPE
            rr += 1
            if True:
                if idx[k] >= len(jl[k]):
                    # finalize stripe k: DMA the min-d2 field; host does
                    # sqrt + AA transfer (clip(thr - dist, 0, 1))
                    items.append(("fin", k, "dma"))
                    finflags[k] = "dma"
                    done[k] = True
                    continue
                s, x0, w, cap = jl[k][idx[k]]
                idx[k] += 1
                if not cap:
                    items.append(("line", k, x0, w, col))
                    fill.append(("line", col, slots[k], s, x0, "v"))
                    acc["V"] += _c_vcus(w)
                    col += 2
                else:
                    # producer: ACT Abs vs V affine(ts)
                    if acc["ACT"] + _c_act(w) <= acc["V"] + 2 * _c_vcus(w):
                        pe = "act"
                        acc["ACT"] += _c_act(w)
                    else:
                        pe = "v"
                        acc["V"] += _c_vcus(w)
                    acc["V"] += _c_vcus(w)
                    hval = _h_for(xy, slots[k], s)
                    items.append(("cap", k, x0, w, col, pe, hval))
                    fill.append(("cap", col, slots[k], s, x0, pe))
                    col += 4
        ncol_max = max(ncol_max, col)
        arms.append(tuple(items))
        fills.append(fill)
        assign.append((tuple(slots), tuple(finflags)))

    struct = (tuple(arms), ncol_max)
    return struct, (fills, assign, ncol_max), thr


def _h_for(xy, pair, s):
    """imm2 cap offset -h = -dn2*s for segment s of image b (fp32-rounded)."""
    b, T = pair
    p0a, p1a, da = _segments(xy[b])
    dx, dy = da[s, 0], da[s, 1]
    dd2 = dx * dx + dy * dy
    return -float(np.float32((dd2 / 2.0) / np.sqrt(dd2)))


# --------------------------------------------------------------------------
# program build (cached per struct)
# --------------------------------------------------------------------------

def _build_program(struct):
    import concourse.tile as tile
    from concourse import bacc, mybir

    dt = mybir.dt
    op = mybir.AluOpType
    af = mybir.ActivationFunctionType
    lmin_op, caphmin_op, e2v_op, sqimin_op = _get_dve_ops()
    arms, ncol = struct

    nc = bacc.Bacc("TRN2", target_bir_lowering=False, debug=False)
    cs_d = nc.dram_tensor("cs", [P, ncol], dt.float32, kind="ExternalInput").ap()
    out_d = nc.dram_tensor(
        "out", [NSTRIPE, P, W], dt.bfloat16, kind="ExternalOutput"
    ).ap()

    with tile.TileContext(nc) as tc, ExitStack() as ctx:
        const = ctx.enter_context(tc.tile_pool(name="const", bufs=1))
        cs = const.tile_from(cs_d)

        mpool = ctx.enter_context(tc.tile_pool(name="m", bufs=1))
        work = ctx.enter_context(tc.tile_pool(name="work", bufs=6))
        opool = ctx.enter_context(tc.tile_pool(name="o", bufs=4))

        # warm the ACT Abs table ASAP (1.5us load, off the critical path)
        wu = opool.tile([P, 8], dt.float32, name="wu")
        nc.vector.memset(wu[:], 0.0)
        wu2 = opool.tile([P, 8], dt.float32, name="wu2")
        nc.scalar.activation(wu2[:], wu[:], af.Abs)

        # x-coordinate row built on device (exact for 0..511 in fp32)
        xt = const.tile([P, W], dt.float32, name="xt")
        nc.gpsimd.iota(
            xt[:], [[1, W]], channel_multiplier=0,
            allow_small_or_imprecise_dtypes=True,
        )
        M = [mpool.tile([P, W], dt.bfloat16, name=f"M{k}") for k in range(NSTRIPE)]
        for k in range(NSTRIPE):
            nc.gpsimd.memset(M[k][:], 1.0e12)

        engs = [mybir.EngineType.DVE, mybir.EngineType.Activation, mybir.EngineType.SP]
        cid = nc.partition_id(engines=engs)
        gi = 0
        for arm in tc.Switch(cid, 8):
            for it in arms[arm]:
                gi += 1
                if it[0] == "line":
                    _, k, x0, w, c = it
                    nc.vector._custom_dve(
                        lmin_op,
                        out=M[k][:, x0 : x0 + w],
                        in0=xt[:, x0 : x0 + w],
                        in1=M[k][:, x0 : x0 + w],
                        s0=cs[:, c : c + 1],
                        s1=cs[:, c + 1 : c + 2],
                    )
                elif it[0] == "cap":
                    _, k, x0, w, c, pe, hval = it
                    if pe == "act":
                        At = work.tile([P, W], dt.float32, tag="At", name=f"A{gi}")
                        nc.scalar.activation(
                            At[:, :w], xt[:, x0 : x0 + w], af.Abs,
                            bias=cs[:, c + 1 : c + 2], scale=cs[:, c : c + 1],
                        )
                        nc.vector._custom_dve(
                            caphmin_op,
                            out=M[k][:, x0 : x0 + w],
                            in0=At[:, :w],
                            in1=M[k][:, x0 : x0 + w],
                            s0=cs[:, c + 2 : c + 3],
                            s1=cs[:, c + 3 : c + 4],
                            imm2=hval,
                        )
                    else:
                        E2 = work.tile([P, W], dt.float32, tag="At", name=f"A{gi}")
                        nc.vector._custom_dve(
                            e2v_op,
                            out=E2[:, :w],
                            in0=xt[:, x0 : x0 + w],
                            s0=cs[:, c : c + 1],
                            s1=cs[:, c + 1 : c + 2],
                            imm2=hval,
                        )
                        nc.vector._custom_dve(
                            sqimin_op,
                            out=M[k][:, x0 : x0 + w],
                            in0=E2[:, :w],
                            in1=M[k][:, x0 : x0 + w],
                            s0=cs[:, c + 2 : c + 3],
                            s1=cs[:, c + 3 : c + 4],
                        )
                else:  # fin: ship the min-d2 field for this stripe
                    _, k, fe = it
                    nc.sync.dma_start(out_d[k, :, :], M[k][:])

    nc.compile()
    return nc


# --------------------------------------------------------------------------
# host coefficient tables
# --------------------------------------------------------------------------

def _prep_inputs(trajectories, struct, planinfo, thr):
    fills, assign, ncol = planinfo
    xy = np.asarray(trajectories, dtype=np.float64)[:, :, 1:3]

    geo = {}
    for b in range(xy.shape[0]):
        p0a, p1a, da = _segments(xy[b])
        dx, dy = da[:, 0], da[:, 1]
        dd2 = dx * dx + dy * dy
        sq = 1.0 / np.sqrt(dd2)
        dn2 = dd2 / 2.0
        c0 = dx * p0a[:, 0] + dy * p0a[:, 1]
        cP = dx * p0a[:, 1] - dy * p0a[:, 0]
        geo[b] = (dx, dy, sq, dn2, c0, cP)

    yv = np.arange(H, dtype=np.float64).reshape(NSTRIPE, P)
    in_maps = []
    for core in range(8):
        cs = np.zeros((P, ncol))
        cs[:, 0] = thr
        for rec in fills[core]:
            kind, c, pair, s, x0, pe = rec
            b, T = pair
            dx, dy, sqv, dn2, c0, cP = geo[b]
            aP = dy[s] * sqv[s]
            bP = (-dx[s] * yv[T] + cP[s]) * sqv[s]
            if kind == "line":
                cs[:, c + 0] = aP
                cs[:, c + 1] = bP  # absolute x via xt
            else:
                aw = dx[s] * sqv[s]
                bw = (dy[s] * yv[T] - (c0[s] + dn2[s])) * sqv[s]
                cs[:, c + 0] = aw
                cs[:, c + 1] = bw  # absolute x via xt (At and E2V)
                cs[:, c + 2] = aP
                # caphmin/sqimin Pp term: scan value at k = C1 + C0*(k+1)
                cs[:, c + 3] = bP + aP * (x0 - 1)
        in_maps.append({"cs": cs.astype(np.float32)})
    return in_maps


def kernel(**inputs):
    from concourse.bass_utils import run_bass_kernel_spmd

    images = np.asarray(inputs["images"])
    trajectories = np.asarray(inputs["trajectories"])
    line_width = inputs["line_width"]
    assert images.shape == (B, C, H, W), images.shape

    struct, planinfo, thr = _plan(trajectories, line_width)
    progs = _state.setdefault("progs", {})
    if struct not in progs:
        progs[struct] = _build_program(struct)
    nc = progs[struct]

    in_maps = _prep_inputs(trajectories, struct, planinfo, thr)
    res = run_bass_kernel_spmd(nc, in_maps, list(range(B))).results
    fills, assign, ncol = planinfo
    out = np.empty((B, C, H, W), np.float32)
    for core in range(B):
        blk = res[core]["out"]  # [NSTRIPE, P, W]
        slots, finflags = assign[core]
        for k in range(NSTRIPE):
            if slots[k] is None:
                continue
            b, T = slots[k]
            d2 = blk[k].astype(np.float32)
            cov = np.clip(thr - np.sqrt(d2), 0.0, 1.0)
            out[b, :, T * P : (T + 1) * P, :] = cov[None, :, :]
    return out


if __name__ == "__main__":
    rng = np.random.default_rng(0)
    ins = {
        "images": rng.standard_normal((B, C, H, W)).astype(np.float32),
        "trajectories": np.concatenate(
            [
                np.broadcast_to(np.linspace(0, 1, K, dtype=np.float32), (B, K))[..., None],
                rng.uniform(0, W - 1, (B, K, 2)).astype(np.float32),
                np.ones((B, K, 1), np.float32),
            ],
            axis=-1,
        ),
        "line_width": 3,
    }
    out = kernel(**ins)
    print(out.shape, out.dtype, out.min(), out.max())


# revision 3
# speedup vs baseline: 1.1298x; 1.0072x over previous
"""Trainium2 Bass kernel v2 for BlittingStrokeModel (AA polyline rasterization).

Reference semantics: per batch item, stamp 16 AA segments onto a zero canvas:
    dist = point-to-segment distance
    cov  = clip(line_width + 0.5 - dist, 0, 1), max over segments, x3 channels.

Device formulation: per 128-row stripe, a bf16 min-accumulator M holds the
running min over segments of squared point-to-segment distance; the host
applies the fixed AA transfer curve cov = clip(thr - sqrt(M), 0, 1) and
broadcasts the (channel-identical) result to 3 channels.

v2 design (vs v1's full-width padded SPMD slots; 65.1us -> 28.3us):
  * Exact per-core programs via tc.Switch(core_id, 8): every op covers only
    the columns its segment can influence (~19% of full width).
  * In-place min accumulation: per stripe one bf16 M tile [128, 512] is
    memset to 1e12 and updated in place by column-ranged fused min ops;
    4 stripes emitted round-robin keep the Vector engine pipelined.
  * Cap (endpoint-clamp) jobs, ACT-producer path (one ACT op + one V op):
        ACT:  At = Abs(aw*x + bw)        (per-partition scale/bias)
        V:    M  = min(Pp^2 + relu(At + imm2)^2, M)   [CAPHSC]
    where Pp = scan(ADD, C0, init=C1) generates the perpendicular-distance
    affine term in one ALU stage (the Idx*C0+C1 form needs 9 > 8 stages)
    and imm2 = -h is the clamp offset baked as an instruction immediate
    (input-specialized program, cached per input).
  * Cap jobs, all-V path (used to balance engines):
        V: E2 = relu(|aw*x + bw| + imm2)^2            [E2V]
        V: M  = min(scan-Pp^2 + E2, M)                [SQIMIN]
  * Line jobs (host proves the endpoint clamp cannot matter inside the
    job window, via exact strip-vs-window polygon clipping): single fused
        V: M = min((x*C0 + C1)^2, M)                  [LD2MIN]
  * x-coordinate row generated on device (gpsimd iota, exact in fp32).
  * Host does fp64 geometry: per-(image, stripe) job detection with exact
    column ranges, LPT stripe->core packing, greedy V/ACT load balancing
    with HW-measured per-op costs (V custom ~1.04w+189ns, ACT ~0.83w+280ns).
  * Output: one bf16 min-d2 stripe field per slot (0.5 MB/core total);
    host does sqrt + transfer curve + channel broadcast + [0,1] clamp.

Images never touch the device (the output is independent of image values).
"""

import numpy as np
from contextlib import ExitStack

B, C, H, W = 8, 3, 512, 512
K = 17
NSEG = K - 1
P = 128
NSTRIPE = H // P  # 4
RMARG = 0.7   # job-range margin (px)
TAU_M = 0.6   # cap-test band margin (px)

# measured per-op costs (ns), back-to-back on HW
def _c_act(w): return 0.833 * w + 280.0
def _c_vcus(w): return 1.042 * w + 189.0
def _c_vts(w): return 0.521 * w + 129.0

_state = {}


# --------------------------------------------------------------------------
# custom DVE ops
# --------------------------------------------------------------------------

def _register_dve_op(name, spec):
    import concourse.dve_ops as dve_ops
    from concourse.dve_ops import DveOp, OPS, _SUB_OPCODE_FOR_NAME, _CUSTOM_DVE_ROW_BASE
    from concourse.dve_spec import lower, _has_src1
    from concourse.dve_uop import DveOpSpec
    from concourse.dve_table_gen import dve_ver_for

    if name in _SUB_OPCODE_FOR_NAME:
        return next(o for o in OPS if o.name == name)
    row = _CUSTOM_DVE_ROW_BASE + len(OPS)
    assert row < 0x20
    _SUB_OPCODE_FOR_NAME[name] = row
    ver = dve_ver_for("TRN2")
    tmp = DveOpSpec(
        name=name, opcode=row, uops=lower(spec, ver=ver), rd1_en=_has_src1(spec)
    )
    op = DveOp(name, spec, subdim=False, uops_sha={ver: tmp.sha(ver)})
    OPS.append(op)
    dve_ops.CUSTOM_DVE_SPECS[name] = spec
    return op


def _get_dve_ops():
    if "ops" in _state:
        return _state["ops"]
    from concourse.dve_spec import (
        Spec, Src0, Src1, C0, C1, C2, sq, minn, maxx, relu, Idx, Zero,
        Scan, Bin, AluOp,
    )

    def _idx1(in0):
        # scan(ADD, C0, init=C1) at element k equals C1 + C0*(k+1)
        return np.arange(in0.shape[-1], dtype=np.float32)[None, :] + 1.0

    _idx1v = _idx1

    lmin = _register_dve_op(
        "STROKE_LD2MIN_ANT",
        Spec(
            body=minn(sq(Src0 * C0 + C1), Src1),
            reference=lambda in0, in1, s0, s1, imm2: np.minimum(
                (in0.astype(np.float32) * s0 + s1) ** 2, in1
            ).astype(np.float32),
        ),
    )
    caphmin = _register_dve_op(
        "STROKE_CAPHSC_ANT",
        Spec(
            body=minn(
                sq(Scan(AluOp.ADD, C0, init=C1)) + sq(relu(Src0 + C2)), Src1
            ),
            reference=lambda in0, in1, s0, s1, imm2: np.minimum(
                (_idx1(in0) * s0 + s1) ** 2
                + np.maximum(in0.astype(np.float32) + imm2, 0.0) ** 2,
                in1,
            ).astype(np.float32),
        ),
    )
    e2v = _register_dve_op(
        "STROKE_E2V_ANT",
        Spec(
            body=sq(relu(Bin(AluOp.ABSOLUTE_VALUE, Src0 * C0 + C1, Zero) + C2)),
            reference=lambda in0, in1, s0, s1, imm2: (
                np.maximum(
                    np.abs(in0.astype(np.float32) * s0 + s1) + imm2, 0.0
                ) ** 2
            ).astype(np.float32),
        ),
    )
    sqimin = _register_dve_op(
        "STROKE_SQIMIN_ANT",
        Spec(
            body=minn(sq(Scan(AluOp.ADD, C0, init=C1)) + Src0, Src1),
            reference=lambda in0, in1, s0, s1, imm2: np.minimum(
                (_idx1(in0) * s0 + s1) ** 2 + in0.astype(np.float32) * 0
                + in0.astype(np.float32),
                in1,
            ).astype(np.float32),
        ),
    )
    _state["ops"] = (lmin, caphmin, e2v, sqimin)
    return _state["ops"]


# --------------------------------------------------------------------------
# host geometry
# --------------------------------------------------------------------------

def _segments(xy):
    """Guarded segment endpoints/deltas (fp64). xy: [K, 2]."""
    p0, p1 = xy[:-1].copy(), xy[1:].copy()
    d = p1 - p0
    degen = (d[:, 0] ** 2 + d[:, 1] ** 2) < 1e-12
    d[degen, 0] = 1e-6
    p1 = p0 + d
    return p0, p1, d


def _clip_poly(poly, a, b, c):
    """Keep the part of polygon with a*x + b*y <= c."""
    out = []
    n = len(poly)
    for i in range(n):
        p, q = poly[i], poly[(i + 1) % n]
        fp = a * p[0] + b * p[1] - c
        fq = a * q[0] + b * q[1] - c
        if fp <= 0:
            out.append(p)
        if (fp < 0) != (fq < 0) and fp != fq:
            t = fp / (fp - fq)
            out.append((p[0] + t * (q[0] - p[0]), p[1] + t * (q[1] - p[1])))
    return out


def _jobs_for(xy, thr):
    """Per stripe: list of (seg, x0, w, needs_cap). fp64 geometry."""
    R = thr + RMARG
    TAU = thr + TAU_M
    FAR = 2000.0
    p0a, p1a, da = _segments(xy)
    out = [[] for _ in range(NSTRIPE)]
    for T in range(NSTRIPE):
        ylo, yhi = T * P + 0.0, T * P + 127.0
        for s in range(NSEG):
            p0, p1, d = p0a[s], p1a[s], da[s]
            y0s, y1s = (p0[1], p1[1]) if p0[1] <= p1[1] else (p1[1], p0[1])
            if y1s < ylo - R or y0s > yhi + R:
                continue
            dy = d[1]
            if abs(dy) < 1e-12:
                t0c, t1c = 0.0, 1.0
            else:
                ta = (ylo - R - p0[1]) / dy
                tb = (yhi + R - p0[1]) / dy
                t0c, t1c = max(0.0, min(ta, tb)), min(1.0, max(ta, tb))
            if t0c > t1c:
                continue
            xs0 = p0[0] + t0c * d[0]
            xs1 = p0[0] + t1c * d[0]
            x0f = max(0.0, min(xs0, xs1) - R)
            x1f = min(W - 1.0, max(xs0, xs1) + R)
            if x1f < x0f:
                continue
            X0, X1 = int(np.floor(x0f)), int(np.ceil(x1f))
            w = X1 - X0 + 1
            # exact endpoint-cap need: does the beyond-endpoint strip
            # (width 2*TAU along the extension ray) intersect the window?
            L = float(np.hypot(*d))
            u = d / L
            nv = np.array([-u[1], u[0]])
            cap = False
            for pe, sgn in ((p1, 1.0), (p0, -1.0)):
                du = u * sgn
                poly = [
                    tuple(pe + TAU * nv), tuple(pe + FAR * du + TAU * nv),
                    tuple(pe + FAR * du - TAU * nv), tuple(pe - TAU * nv),
                ]
                poly = _clip_poly(poly, 1, 0, X1 + 0.5)
                if poly:
                    poly = _clip_poly(poly, -1, 0, -(X0 - 0.5))
                if poly:
                    poly = _clip_poly(poly, 0, 1, yhi + 0.5)
                if poly:
                    poly = _clip_poly(poly, 0, -1, -(ylo - 0.5))
                if poly:
                    cap = True
                    break
            out[T].append((s, X0, w, cap))
    return out


# --------------------------------------------------------------------------
# planning: stripe->core packing + per-core schedule & engine balance
# --------------------------------------------------------------------------

def _plan(trajectories, line_width):
    thr = float(np.asarray(line_width).item()) + 0.5
    xy = np.asarray(trajectories, dtype=np.float64)[:, :, 1:3]
    nb = xy.shape[0]

    jobs = {}
    for b in range(nb):
        jb = _jobs_for(xy[b], thr)
        for T in range(NSTRIPE):
            jobs[(b, T)] = jb[T]

    def stripe_cost(jl):
        c = 0.0
        for s, x0, w, cap in jl:
            c += _c_vcus(w) + (_c_act(w) if cap else 0.0) * 0.5
        return c

    pairs = sorted(jobs, key=lambda p: -stripe_cost(jobs[p]))
    cores = [[] for _ in range(8)]
    loads = [0.0] * 8
    for pr in pairs:
        cand = [c for c in range(8) if len(cores[c]) < NSTRIPE]
        i = min(cand, key=lambda c: loads[c])
        cores[i].append(pr)
        loads[i] += stripe_cost(jobs[pr])

    # per-core schedule: round-robin stripes, greedy engine choice
    arms = []     # structural, hashable
    fills = []    # per-core value-filling recipes
    assign = []   # per-core [(b, T)] per slot + finsub engine flags
    ncol_max = 1
    for core in range(8):
        slots = cores[core]
        # order slots by descending job count so round-robin staggers finishes
        slots = sorted(slots, key=lambda p: -len(jobs[p]))
        while len(slots) < NSTRIPE:
            slots.append(None)
        jl = [list(jobs[p]) if p is not None else [] for p in slots]
        # line jobs first (V-only, no ACT dep) then caps, wide first
        for l in jl:
            l.sort(key=lambda j: (j[3], -j[2]))
        items = []   # structural items
        fill = []    # (kind, col, values...) value recipes
        # seed with startup costs: ACT table load + warmup; V memset wait
        acc = {"V": 300.0, "ACT": 1900.0}
        col = 1      # col 0 = thr
        idx = [0] * NSTRIPE
        done = [False] * NSTRIPE
        finflags = [None] * NSTRIPE
        rr = 0
        while not all(done):
            # plain round-robin across unfinished stripes (4 independent
            # chains keep V fed; equal finish keeps the critical path short)
            while done[rr % NSTRIPE]:
                rr += 1
            k = rr % NSTRIPE
            rr += 1
            if True:
                if idx[k] >= len(jl[k]):
                    # finalize stripe k: DMA the min-d2 field; host does
                    # sqrt + AA transfer (clip(thr - dist, 0, 1))
                    items.append(("fin", k, "dma"))
                    finflags[k] = "dma"
                    done[k] = True
                    continue
                s, x0, w, cap = jl[k][idx[k]]
                idx[k] += 1
                if not cap:
                    items.append(("line", k, x0, w, col))
                    fill.append(("line", col, slots[k], s, x0, "v"))
                    acc["V"] += _c_vcus(w)
                    col += 2
                else:
                    # producer: ACT Abs vs V affine(ts)
                    if acc["ACT"] + _c_act(w) <= acc["V"] + 2 * _c_vcus(w):
                        pe = "act"
                        acc["ACT"] += _c_act(w)
                    else:
                        pe = "v"
                        acc["V"] += _c_vcus(w)
                    acc["V"] += _c_vcus(w)
                    hval = _h_for(xy, slots[k], s)
                    items.append(("cap", k, x0, w, col, pe, hval))
                    fill.append(("cap", col, slots[k], s, x0, pe))
                    col += 4
        ncol_max = max(ncol_max, col)
        arms.append(tuple(items))
        fills.append(fill)
        assign.append((tuple(slots), tuple(finflags)))

    struct = (tuple(arms), ncol_max)
    return struct, (fills, assign, ncol_max), thr


def _h_for(xy, pair, s):
    """imm2 cap offset -h = -dn2*s for segment s of image b (fp32-rounded)."""
    b, T = pair
    p0a, p1a, da = _segments(xy[b])
    dx, dy = da[s, 0], da[s, 1]
    dd2 = dx * dx + dy * dy
    return -float(np.float32((dd2 / 2.0) / np.sqrt(dd2)))


# --------------------------------------------------------------------------
# program build (cached per struct)
# --------------------------------------------------------------------------

def _build_program(struct):
    import concourse.tile as tile
    from concourse import bacc, mybir

    dt = mybir.dt
    op = mybir.AluOpType
    af = mybir.ActivationFunctionType
    lmin_op, caphmin_op, e2v_op, sqimin_op = _get_dve_ops()
    arms, ncol = struct

    nc = bacc.Bacc("TRN2", target_bir_lowering=False, debug=False)
    cs_d = nc.dram_tensor("cs", [P, ncol], dt.float32, kind="ExternalInput").ap()
    out_d = nc.dram_tensor(
        "out", [NSTRIPE, P, W], dt.bfloat16, kind="ExternalOutput"
    ).ap()

    with tile.TileContext(nc) as tc, ExitStack() as ctx:
        const = ctx.enter_context(tc.tile_pool(name="const", bufs=1))
        cs = const.tile_from(cs_d)

        mpool = ctx.enter_context(tc.tile_pool(name="m", bufs=1))
        work = ctx.enter_context(tc.tile_pool(name="work", bufs=6))
        opool = ctx.enter_context(tc.tile_pool(name="o", bufs=4))

        # warm the ACT Abs table ASAP (1.5us load, off the critical path)
        wu = opool.tile([P, 8], dt.float32, name="wu")
        nc.vector.memset(wu[:], 0.0)
        wu2 = opool.tile([P, 8], dt.float32, name="wu2")
        nc.scalar.activation(wu2[:], wu[:], af.Abs)

        # x-coordinate row built on device (exact for 0..511 in fp32)
        xt = const.tile([P, W], dt.float32, name="xt")
        nc.gpsimd.iota(
            xt[:], [[1, W]], channel_multiplier=0,
            allow_small_or_imprecise_dtypes=True,
        )
        M = [mpool.tile([P, W], dt.bfloat16, name=f"M{k}") for k in range(NSTRIPE)]
        for k in range(NSTRIPE):
            nc.gpsimd.memset(M[k][:], 1.0e12)

        engs = [mybir.EngineType.DVE, mybir.EngineType.Activation, mybir.EngineType.SP]
        cid = nc.partition_id(engines=engs)
        gi = 0
        for arm in tc.Switch(cid, 8):
            for it in arms[arm]:
                gi += 1
                if it[0] == "line":
                    _, k, x0, w, c = it
                    nc.vector._custom_dve(
                        lmin_op,
                        out=M[k][:, x0 : x0 + w],
                        in0=xt[:, x0 : x0 + w],
                        in1=M[k][:, x0 : x0 + w],
                        s0=cs[:, c : c + 1],
                        s1=cs[:, c + 1 : c + 2],
                    )
                elif it[0] == "cap":
                    _, k, x0, w, c, pe, hval = it
                    if pe == "act":
                        At = work.tile([P, W], dt.float32, tag="At", name=f"A{gi}")
                        nc.scalar.activation(
                            At[:, :w], xt[:, x0 : x0 + w], af.Abs,
                            bias=cs[:, c + 1 : c + 2], scale=cs[:, c : c + 1],
                        )
                        nc.vector._custom_dve(
                            caphmin_op,
                            out=M[k][:, x0 : x0 + w],
                            in0=At[:, :w],
                            in1=M[k][:, x0 : x0 + w],
                            s0=cs[:, c + 2 : c + 3],
                            s1=cs[:, c + 3 : c + 4],
                            imm2=hval,
                        )
                    else:
                        E2 = work.tile([P, W], dt.float32, tag="At", name=f"A{gi}")
                        nc.vector._custom_dve(
                            e2v_op,
                            out=E2[:, :w],
                            in0=xt[:, x0 : x0 + w],
                            s0=cs[:, c : c + 1],
                            s1=cs[:, c + 1 : c + 2],
                            imm2=hval,
                        )
                        nc.vector._custom_dve(
                            sqimin_op,
                            out=M[k][:, x0 : x0 + w],
                            in0=E2[:, :w],
                            in1=M[k][:, x0 : x0 + w],
                            s0=cs[:, c + 2 : c + 3],
                            s1=cs[:, c + 3 : c + 4],
                        )
                else:  # fin: ship the min-d2 field for this stripe
                    _, k, fe = it
                    nc.sync.dma_start(out_d[k, :, :], M[k][:])

    nc.compile()
    return nc


# --------------------------------------------------------------------------
# host coefficient tables
# --------------------------------------------------------------------------

def _prep_inputs(trajectories, struct, planinfo, thr):
    fills, assign, ncol = planinfo
    xy = np.asarray(trajectories, dtype=np.float64)[:, :, 1:3]

    geo = {}
    for b in range(xy.shape[0]):
        p0a, p1a, da = _segments(xy[b])
        dx, dy = da[:, 0], da[:, 1]
        dd2 = dx * dx + dy * dy
        sq = 1.0 / np.sqrt(dd2)
        dn2 = dd2 / 2.0
        c0 = dx * p0a[:, 0] + dy * p0a[:, 1]
        cP = dx * p0a[:, 1] - dy * p0a[:, 0]
        geo[b] = (dx, dy, sq, dn2, c0, cP)

    yv = np.arange(H, dtype=np.float64).reshape(NSTRIPE, P)
    in_maps = []
    for core in range(8):
        cs = np.zeros((P, ncol))
        cs[:, 0] = thr
        for rec in fills[core]:
            kind, c, pair, s, x0, pe = rec
            b, T = pair
            dx, dy, sqv, dn2, c0, cP = geo[b]
            aP = dy[s] * sqv[s]
            bP = (-dx[s] * yv[T] + cP[s]) * sqv[s]
            if kind == "line":
                cs[:, c + 0] = aP
                cs[:, c + 1] = bP  # absolute x via xt
            else:
                aw = dx[s] * sqv[s]
                bw = (dy[s] * yv[T] - (c0[s] + dn2[s])) * sqv[s]
                cs[:, c + 0] = aw
                cs[:, c + 1] = bw  # absolute x via xt (At and E2V)
                cs[:, c + 2] = aP
                # caphmin/sqimin Pp term: scan value at k = C1 + C0*(k+1)
                cs[:, c + 3] = bP + aP * (x0 - 1)
        in_maps.append({"cs": cs.astype(np.float32)})
    return in_maps


def kernel(**inputs):
    from concourse.bass_utils import run_bass_kernel_spmd

    images = np.asarray(inputs["images"])
    trajectories = np.asarray(inputs["trajectories"])
    line_width = inputs["line_width"]
    assert images.shape == (B, C, H, W), images.shape

    struct, planinfo, thr = _plan(trajectories, line_width)
    progs = _state.setdefault("progs", {})
    if struct not in progs:
        progs[struct] = _build_program(struct)
    nc = progs[struct]

    in_maps = _prep_inputs(trajectories, struct, planinfo, thr)
    res = run_bass_kernel_spmd(nc, in_maps, list(range(B))).results
    fills, assign, ncol = planinfo
    out = np.empty((B, C, H, W), np.float32)
    for core in range(B):
        blk = res[core]["out"]  # [NSTRIPE, P, W]
        slots, finflags = assign[core]
        for k in range(NSTRIPE):
            if slots[k] is None:
                continue
            b, T = slots[k]
            d2 = blk[k].astype(np.float32)
            cov = np.clip(thr - np.sqrt(d2), 0.0, 1.0)
            out[b, :, T * P : (T + 1) * P, :] = cov[None, :, :]
    return out


if __name__ == "__main__":
    rng = np.random.default_rng(0)
    ins = {
        "images": rng.standard_normal((B, C, H, W)).astype(np.float32),
        "trajectories": np.concatenate(
            [
                np.broadcast_to(np.linspace(0, 1, K, dtype=np.float32), (B, K))[..., None],
                rng.uniform(0, W - 1, (B, K, 2)).astype(np.float32),
                np.ones((B, K, 1), np.float32),
            ],
            axis=-1,
        ),
        "line_width": 3,
    }
    out = kernel(**ins)
    print(out.shape, out.dtype, out.min(), out.max())
